# revision 8
# baseline (speedup 1.0000x reference)
"""Multi-head attention (D=2048, H=16, B=2, S=2048, causal, RoPE) on 8 TRN2 cores.

Sharding: tensor-parallel over heads -- 2 heads per core, both batches.
Each core computes q/k/v projections for its 2 heads, RoPE, causal flash-style
attention, and a partial output projection over its heads' columns of wo.
The host sums the 8 partial outputs (the out-projection contracts over heads,
which is the sharded axis).

Schedule: batch-0 projections first, then batch-1 projections interleaved
with batch-0 attention; out-projections spread as fillers inside attention
chunks. A PE warmup (dummy matmuls while the first DMAs stream) keeps the
tensor engine's p-state clock ramped. Output partials are written bf16
(host sums 8 partials in f32): halves output DMA bytes for ~3e-4 extra
max-rel error. A deep bf16 eviction ring (KOUTP=16) keeps the tail drain
off the critical path.

Self-contained: hardcodes all shapes; only needs numpy/ml_dtypes/concourse.
"""
import os
import sys
import time

for _p in ("/opt/trn_rl_repo",):
    if os.path.isdir(_p) and _p not in sys.path:
        sys.path.append(_p)

import numpy as np
import ml_dtypes
from contextlib import ExitStack

import concourse.bass as bass
import concourse.tile as tile
from concourse import bacc, mybir

BF = mybir.dt.bfloat16
F32 = mybir.dt.float32
F32R = mybir.dt.float32r
BF_NP = ml_dtypes.bfloat16

B = 2
S = 2048
D = 2048
H = 16
HD = 128  # head dim
N_CORES = 8
H_CORE = H // N_CORES          # heads per core = 2
E = H_CORE * HD                # per-core q/k/v width = 256
BS = B * S                     # 4096 flattened tokens
P = 128
SC = 512                       # s-chunk (free dim of projection matmuls)
N_SC = BS // SC                # 8 s-chunks
N_DT = D // P                  # 16 d-tiles (contraction)
QC = 512                       # q-chunk in attention
N_QC = S // QC                 # 4 q-chunks per (batch, head)
N_KT = S // P                  # 16 k-tiles per (batch, head)
SCALE = 1.0 / float(np.sqrt(HD))
ROPE_BASE = 10000.0


def _build_program():
    """Build the per-core Bass program (identical on all cores; data differs)."""
    nc = bacc.Bacc("TRN2", target_bir_lowering=False, debug=False)

    # all big inputs are host-packed to the exact SBUF layout so every DMA is
    # one long contiguous run per partition (few descriptors, fast HWDGE)
    xt_d = nc.dram_tensor("xt", [N_SC, P, N_DT * SC], BF, kind="ExternalInput").ap()
    wqt_d = nc.dram_tensor("wqt", [P, N_DT * E], BF, kind="ExternalInput").ap()
    wkt_d = nc.dram_tensor("wkt", [P, N_DT * E], BF, kind="ExternalInput").ap()
    wvt_d = nc.dram_tensor("wvt", [P, N_DT * E], BF, kind="ExternalInput").ap()
    wot_d = nc.dram_tensor("wot", [P, H_CORE * D], BF, kind="ExternalInput").ap()
    cos_d = nc.dram_tensor("cos", [P, S], BF, kind="ExternalInput").ap()
    sin_d = nc.dram_tensor("sin", [P, S], BF, kind="ExternalInput").ap()
    rmat_d = nc.dram_tensor("rmat", [P, P], BF, kind="ExternalInput").ap()
    tri_d = nc.dram_tensor("tri", [P, P], BF, kind="ExternalInput").ap()
    obf = os.environ.get("KOBF", "1") == "1"
    out_d = nc.dram_tensor("out", [BS, D], BF if obf else F32,
                           kind="ExternalOutput").ap()

    with tile.TileContext(nc) as tc:
        with ExitStack() as ctx:
            _emit(ctx, tc, nc, xt_d, wqt_d, wkt_d, wvt_d, wot_d,
                  cos_d, sin_d, rmat_d, tri_d, out_d)
    nc.compile()
    return nc


def _emit(ctx, tc, nc, xt_d, wqt_d, wkt_d, wvt_d, wot_d,
          cos_d, sin_d, rmat_d, tri_d, out_d):
    Exp = mybir.ActivationFunctionType.Exp
    # NOTE: Pool (gpsimd) measures ~1155ns per [128,512] tensor op on hw --
    # ~3x the DVE-class model -- so accumulating softmax denominators there
    # stalls PE ~3.8us per chunk. DVE measures ~134-250ns for a bf16 add, so
    # den lives there (KDEN=dve): acc += at per k-tile, one ones-matmul per
    # chunk for the partition reduction. Each acc element sums <=16 bf16
    # values so the rounding is ~0.5% on acc, ~0.04% after the 128-way f32
    # partition sum -- negligible vs the 2e-2 gate.
    den_mode = os.environ.get("KDEN", "dve")
    use_pool_den = den_mode == "pool"
    use_dve_den = den_mode == "dve"
    kilv = os.environ.get("KILV", "4")
    kwarm = int(os.environ.get("KWARM", "12"))
    xtq_scalar = os.environ.get("KXTQ", "sync") == "scalar"

    const = ctx.enter_context(tc.tile_pool(name="const", bufs=1))
    xpool = ctx.enter_context(tc.tile_pool(name="xpool", bufs=int(os.environ.get("KXP","2"))))
    qkv = ctx.enter_context(tc.tile_pool(name="qkv", bufs=1))
    rope = ctx.enter_context(tc.tile_pool(name="rope", bufs=int(os.environ.get("KROPE","4"))))
    att = ctx.enter_context(tc.tile_pool(name="att", bufs=8))
    nrm = ctx.enter_context(tc.tile_pool(name="nrm", bufs=int(os.environ.get("KNRM","4"))))
    den = ctx.enter_context(tc.tile_pool(name="den", bufs=2))
    outp = ctx.enter_context(tc.tile_pool(name="outp", bufs=int(os.environ.get("KOUTP","16"))))
    psum = ctx.enter_context(tc.tile_pool(name="psum", bufs=int(os.environ.get("KACC","5")), space="PSUM"))
    psum_s = ctx.enter_context(tc.tile_pool(name="psum_s", bufs=int(os.environ.get("KSTR","3")), space="PSUM"))

    # ---- constants / weights in SBUF ----
    # input queue: xt rides one HWDGE queue, weights the other, outputs go to
    # the weights' queue later (sync) so the xt stream never queues behind
    # output bursts
    xt_eng = nc.scalar if xtq_scalar else nc.sync
    w_eng = nc.sync if xtq_scalar else nc.scalar

    xt_c0 = xpool.tile([P, N_DT * SC], BF, tag="xt")
    wq_sb = const.tile([P, N_DT * E], BF)
    wk_sb = const.tile([P, N_DT * E], BF)
    wv_sb = const.tile([P, N_DT * E], BF)
    # interleave xt0/wq pieces across the two queues, finest first so the
    # first accumulation group starts as soon as d-tile 0 lands
    xt_pieces = [(0, 1), (1, 2), (2, 4), (4, 8), (8, 16)]
    for t0_, t1_ in xt_pieces:
        xt_eng.dma_start(xt_c0[:, t0_ * SC:t1_ * SC],
                         xt_d[0][:, t0_ * SC:t1_ * SC])
        w_eng.dma_start(wq_sb[:, t0_ * E:t1_ * E],
                        wqt_d[:, t0_ * E:t1_ * E])
    rmat_sb = const.tile([P, P], BF)
    tri_sb = const.tile([P, P], BF)
    w_eng.dma_start(rmat_sb[:], rmat_d[:])
    w_eng.dma_start(tri_sb[:], tri_d[:])
    w_eng.dma_start(wk_sb[:], wkt_d[:])
    cos_sb = const.tile([P, S], BF)
    sin_sb = const.tile([P, S], BF)
    w_eng.dma_start(cos_sb[:], cos_d[:])
    w_eng.dma_start(sin_sb[:], sin_d[:])
    w_eng.dma_start(wv_sb[:], wvt_d[:])
    ones_sb = const.tile([P, P], BF)
    # memset on Pool: it is the earliest-active engine (the framework's own
    # const-AP memsets run there in the preamble), so the PE warmup below can
    # start ~2us sooner than if gated on DVE's slower preamble
    warm_eng = nc.gpsimd if os.environ.get("KWME", "vector") == "pool" else nc.vector
    warm_eng.memset(ones_sb[:], 1.0)
    # wot in [128, 2 * D] packed layout; needed only for out-projection
    wo_sb = const.tile([P, H_CORE * D], BF)
    w_eng.dma_start(wo_sb[:], wot_d[:])

    # PE warmup: dummy matmuls ramp the tensor-engine p-state clock while the
    # first xt/weight DMAs are still streaming in
    if kwarm:
        warm_sb = const.tile([P, SC], BF)
        warm_eng.memset(warm_sb[:], 0.0)
        warm_ps = psum.tile([P, SC], F32, tag="ps")
        for i in range(kwarm):
            nc.tensor.matmul(warm_ps[:], ones_sb[:], warm_sb[:],
                             start=True, stop=True)
        # give the warmup PSUM a reader (BIR verifier requires one; Pool
        # cannot read PSUM, so use DVE -- idle at startup)
        nc.vector.tensor_copy(warm_sb[:], warm_ps[:])

    # persistent activations
    qT = qkv.tile([P, H_CORE * BS], BF)   # [d, (head, b*s)] rope'd q
    kT = qkv.tile([P, H_CORE * BS], BF)   # [d, (head, b*s)] rope'd k
    v_sb = qkv.tile([P, (BS // P) * E], BF)  # [s within tile, (s-tile, e)]
    aoT = qkv.tile([P, H_CORE * BS], BF)  # [d, (b, head, q)] normalized attn out

    # ---- phase 1: projections + RoPE ----
    def emit_phase1(sc, xt_c):
        b = sc // (N_SC // B)
        s_lo = (sc % (N_SC // B)) * SC  # within-batch s offset

        # qT / kT (with RoPE) per head (e-tile == head)
        for w_sb, dstT in ((wq_sb, qT), (wk_sb, kT)):
            for h in range(H_CORE):
                pp = psum.tile([P, SC], F32, tag="ps")
                for t in range(N_DT):
                    nc.tensor.matmul(
                        pp[:],
                        w_sb[:, t * E + h * HD: t * E + h * HD + HD],
                        xt_c[:, t * SC:(t + 1) * SC],
                        start=(t == 0), stop=(t == N_DT - 1))
                raw = rope.tile([P, SC], BF, tag="raw")
                nc.scalar.copy(raw[:], pp[:])
                rot = psum_s.tile([P, SC], F32, tag="pss")
                nc.tensor.matmul(rot[:], rmat_sb[:], raw[:], start=True, stop=True)
                t1 = rope.tile([P, SC], BF, tag="t1")
                # raw * cos is SBUF-only: run it on the lightly-used Pool
                nc.gpsimd.tensor_mul(t1[:], raw[:], cos_sb[:, s_lo:s_lo + SC])
                t2 = rope.tile([P, SC], BF, tag="t2")
                nc.vector.tensor_mul(t2[:], rot[:], sin_sb[:, s_lo:s_lo + SC])
                dst = dstT[:, h * BS + sc * SC: h * BS + (sc + 1) * SC]
                nc.vector.tensor_add(dst, t1[:], t2[:])

        # v for this s-chunk: 4 s-subtiles of 128, two per PSUM tile so each
        # eviction copy covers 512 columns
        for sp in range(SC // P // 2):
            pv = psum.tile([P, SC], F32, tag="ps")
            for half in range(2):
                st = sp * 2 + half
                for t in range(N_DT):
                    nc.tensor.matmul(
                        pv[:, half * E:(half + 1) * E],
                        xt_c[:, t * SC + st * P: t * SC + (st + 1) * P],
                        wv_sb[:, t * E:(t + 1) * E],
                        start=(t == 0), stop=(t == N_DT - 1))
            g_st = sc * (SC // P) + sp * 2  # global s-tile index
            nc.scalar.copy(v_sb[:, g_st * E:(g_st + 2) * E], pv[:])

    # ---- attention per (batch, head); the softmax denominator is summed on
    # the Pool engine (KDEN=pool) so PE only does scores + AV ----
    # finishers: each chunk's reciprocal+normalize is deferred into the NEXT
    # chunk (emitted after its first step) so the chunk-end serial chain
    # den->recip->normalize on DVE never blocks the next chunk's exp/AV
    pend_fin = []

    def attention_chunk(b, h, qc, fillers=()):
        # fillers: closures emitted at evenly spaced points of the k-loop
        # (used to spread out-projection work so its PSUM use and eviction
        # load drain gradually instead of in one burst)
        fillers = list(fillers)
        qk_off = h * BS + b * S  # column offset into qT/kT
        out_ps = psum.tile([P, QC], F32, tag="ps")
        if not (use_pool_den or use_dve_den):
            den_ps = psum.tile([P, QC], F32, tag="ps")
        nkt = (qc + 1) * (QC // P)
        ndiag = qc * (QC // P)  # number of full (below-diagonal) k-tiles
        if os.environ.get("KPAIR", "0") == "1":
            steps = [(j, j + 1) for j in range(0, ndiag - 1, 2)]
            if ndiag % 2:
                steps.append((ndiag - 1,))
        else:
            steps = [(j,) for j in range(ndiag)]
        steps += [(j,) for j in range(ndiag, nkt)]
        fill_at = {max(0, ((i + 1) * len(steps)) // len(fillers) - 1): f
                   for i, f in enumerate(fillers)} if fillers else {}
        acc = None
        for si, js in enumerate(steps):
            pair = len(js) == 2
            at = att.tile([P, 2 * QC] if pair else [P, QC], BF,
                          tag="at2" if pair else "at", bufs=int(os.environ.get("KAT","4")))
            sc_ps = psum_s.tile([P, 2 * QC] if pair else [P, QC], F32,
                                tag="pss")
            di = js[0] - ndiag
            q0 = max(di, 0) * P  # valid q suffix start (0 for paired tiles)
            for i, j in enumerate(js):
                nc.tensor.matmul(
                    sc_ps[:, i * QC + q0:(i + 1) * QC],
                    kT[:, qk_off + j * P: qk_off + (j + 1) * P],
                    qT[:, qk_off + qc * QC + q0: qk_off + (qc + 1) * QC],
                    start=True, stop=True)
            nc.scalar.activation(at[:, q0:], sc_ps[:, q0:], Exp, scale=SCALE)
            if di >= 0:
                tri_eng = nc.gpsimd if os.environ.get("KTRI") == "pool" else nc.vector
                tri_eng.tensor_mul(at[:, q0:q0 + P],
                                   at[:, q0:q0 + P], tri_sb[:])
            for i, j in enumerate(js):
                g_st = b * (S // P) + j
                nc.tensor.matmul(
                    out_ps[:, q0:QC],
                    v_sb[:, g_st * E + h * HD: g_st * E + (h + 1) * HD],
                    at[:, i * QC + q0:(i + 1) * QC],
                    start=(j == 0), stop=(j == nkt - 1))
                if not (use_pool_den or use_dve_den):
                    nc.tensor.matmul(
                        den_ps[:, q0:QC],
                        ones_sb[:],
                        at[:, i * QC + q0:(i + 1) * QC],
                        start=(j == 0), stop=(j == nkt - 1))
            if use_dve_den:
                # bf16 running sum of the exp tiles on DVE; the 128-way
                # partition reduction stays in one f32 matmul per chunk
                for i, j in enumerate(js):
                    q0j = max(j - ndiag, 0) * P
                    seg = at[:, i * QC + q0j:(i + 1) * QC]
                    if acc is None:
                        acc = den.tile([P, QC], BF, tag="accd")
                        if q0j == 0:
                            nc.vector.tensor_copy(acc[:], seg)
                        else:
                            nc.vector.memset(acc[:], 0.0)
                            nc.vector.tensor_add(acc[:, q0j:], acc[:, q0j:], seg)
                    else:
                        nc.vector.tensor_add(acc[:, q0j:], acc[:, q0j:], seg)
            elif use_pool_den:
                for i, j in enumerate(js):
                    q0j = max(j - ndiag, 0) * P
                    seg = at[:, i * QC + q0j:(i + 1) * QC]
                    if acc is None:
                        acc = den.tile([P, QC], F32, tag="acc")
                        if q0j == 0:
                            nc.gpsimd.tensor_copy(acc[:], seg)
                        else:
                            nc.gpsimd.memset(acc[:], 0.0)
                            nc.gpsimd.tensor_add(acc[:, q0j:], acc[:, q0j:], seg)
                    else:
                        nc.gpsimd.tensor_add(acc[:, q0j:], acc[:, q0j:], seg)
            if si == 0 and os.environ.get("KFIN", "0") == "1":
                for fin in pend_fin:
                    fin()
                pend_fin.clear()
            if si in fill_at:
                fill_at[si]()

        def finish(out_ps=out_ps, b=b, h=h, qc=qc, acc=acc):
            if use_dve_den:
                dps = psum_s.tile([P, QC], F32, tag="pss")
                nc.tensor.matmul(dps[:], ones_sb[:], acc[:],
                                 start=True, stop=True)
            elif use_pool_den:
                # single bf16 rounding of the final sums (+-0.2% on den), then
                # a 1-cycle/row bf16 matmul does the partition reduction
                acc_bf = den.tile([P, QC], BF, tag="accb")
                nc.gpsimd.tensor_copy(acc_bf[:], acc[:])
                dps = psum_s.tile([P, QC], F32, tag="pss")
                nc.tensor.matmul(dps[:], ones_sb[:], acc_bf[:],
                                 start=True, stop=True)
            else:
                dps = den_ps
            rec = nrm.tile([P, QC], F32, tag="rec")
            nc.vector.reciprocal_approx_fast(rec[:], dps[:])
            dst = aoT[:, (b * H_CORE + h) * S + qc * QC:
                      (b * H_CORE + h) * S + (qc + 1) * QC]
            nc.vector.tensor_mul(dst, out_ps[:], rec[:])
        if os.environ.get("KFIN", "0") == "1":
            pend_fin.append(finish)
        else:
            finish()

    obf = os.environ.get("KOBF", "1") == "1"

    def outproj_st(b, st, tail=False):
        # one 128-row slab of batch b's output, all 4 e-chunks; partials are
        # written bf16 (KOBF=1) -- the host sums 8 partials in f32, so the
        # single rounding costs ~4e-4 max-rel while halving output DMA bytes
        for ec in range(D // SC):
            po = psum.tile([P, SC], F32, tag="ps")
            for h in range(H_CORE):
                lhsT = aoT[:, (b * H_CORE + h) * S + st * P:
                           (b * H_CORE + h) * S + (st + 1) * P]
                nc.tensor.matmul(
                    po[:],
                    lhsT,
                    wo_sb[:, h * D + ec * SC: h * D + (ec + 1) * SC],
                    start=(h == 0), stop=(h == H_CORE - 1))
            o_sb = outp.tile([P, SC], BF if obf else F32, tag="o")
            if tail and ec % 2 == 0:
                # at the tail ACT is idle; otherwise keep ACT exp-only so
                # evictions never delay the exp critical path
                nc.scalar.copy(o_sb[:], po[:])
            else:
                nc.vector.tensor_copy(o_sb[:], po[:])
            dma_eng = nc.scalar if (tail and ec % 2 == 1) else nc.sync
            dma_eng.dma_start(
                out_d[b * S + st * P: b * S + (st + 1) * P,
                      ec * SC:(ec + 1) * SC],
                o_sb[:])

    def outproj_fillers(b, qc, tail=False):
        return [lambda st=st: outproj_st(b, st, tail)
                for st in range(qc * (QC // P), (qc + 1) * (QC // P))]

    xt_split = os.environ.get("KXTS", "0") == "1"

    def load_xt_chunk(sc, eng, pieces=4):
        # KXTS=1: alternate pieces across both HWDGE queues -- phase 1 is
        # xt-supply-limited and the weights queue is idle after startup
        xt_c = xpool.tile([P, N_DT * SC], BF, tag="xt")
        step = N_DT // pieces
        for pi, t0_ in enumerate(range(0, N_DT, step)):
            e = (nc.scalar if pi % 2 else nc.sync) if xt_split else eng
            e.dma_start(xt_c[:, t0_ * SC:(t0_ + step) * SC],
                        xt_d[sc][:, t0_ * SC:(t0_ + step) * SC])
        return xt_c

    def drive():
        if kilv == "4":
            # fully interleaved: attention unit u (= (b, qc)) runs right
            # after projection chunk b*4+qc exists, so ACT/DVE-heavy
            # attention overlaps PE-heavy projections for the whole kernel
            # instead of piling up in a pure-attention endgame. outproj(u)
            # fillers ride inside unit u+1's chunks; the 2nd-to-last unit's
            # outproj is emitted inside the last p1 iter (which still has
            # projection work to hide its eviction load), keeping the final
            # iter's DVE load under PE.
            units = [(0, qc) for qc in range(N_QC)] + \
                    [(1, qc) for qc in range(N_QC)]
            emit_phase1(0, xt_c0)
            for r in range(1, N_SC + 1):
                xt_c = None
                if r < N_SC:
                    xt_c = load_xt_chunk(r, xt_eng)
                b, qc = units[r - 1]
                if r >= 2 and r < N_SC:
                    f = outproj_fillers(*units[r - 2])
                    f1, f2 = f[:2], f[2:]
                else:
                    f1, f2 = [], []
                attention_chunk(b, 0, qc, fillers=f1)
                attention_chunk(b, 1, qc, fillers=f2)
                if r == N_SC:
                    break
                if r == N_SC - 1:
                    # 2nd-to-last unit's outproj before the final p1 chunk
                    # (hidden under its projection load) so the last iter's
                    # DVE sees only den/norm work
                    for fl in outproj_fillers(*units[r - 1]):
                        fl()
                emit_phase1(r, xt_c)
            for fin in pend_fin:
                fin()
            pend_fin.clear()
            for fl in outproj_fillers(B - 1, N_QC - 1, tail=True):
                fl()
            return
        if kilv == "2":
            # attention unit (b, qc) r-1 is ready after projection chunk r-1;
            # weave it before projection chunk r so PE always has DMA-free
            # work while the next x chunk streams in
            units = [(b, qc) for b in range(B) for qc in range(N_QC)]
            emit_phase1(0, xt_c0)
            for r in range(1, N_SC + 1):
                xt_c = None
                if r < N_SC:
                    xt_c = load_xt_chunk(r, xt_eng)
                b, qc = units[r - 1]
                f = outproj_fillers(*units[r - 2]) if r >= 2 else []
                attention_chunk(b, 0, qc)
                attention_chunk(b, 1, qc, fillers=f)
                if r < N_SC:
                    emit_phase1(r, xt_c)
        elif kilv == "1":
            # batch 0 projections first, then batch 1 projections interleaved
            # with batch 0 attention
            for sc in range(N_SC // B):
                emit_phase1(sc, xt_c0 if sc == 0 else load_xt_chunk(sc, nc.sync))
            for qc in range(N_QC):
                emit_phase1(N_SC // B + qc, load_xt_chunk(N_SC // B + qc, nc.sync))
                attention_chunk(0, 0, qc)
                prev = (0, qc - 1)
                fillers = outproj_fillers(*prev) if prev[1] >= 0 else ()
                attention_chunk(0, 1, qc, fillers=fillers)
            for qc in range(N_QC):
                attention_chunk(1, 0, qc)
                prev = (1, qc - 1) if qc > 0 else (0, N_QC - 1)
                attention_chunk(1, 1, qc, fillers=outproj_fillers(*prev))
        else:
            for sc in range(N_SC):
                emit_phase1(sc, xt_c0 if sc == 0 else load_xt_chunk(sc, nc.sync))
            for b in range(B):
                for qc in range(N_QC):
                    attention_chunk(b, 0, qc)
                    prev = (b, qc - 1) if qc > 0 else (b - 1, N_QC - 1)
                    fillers = outproj_fillers(*prev) if prev[0] >= 0 else ()
                    attention_chunk(b, 1, qc, fillers=fillers)
        for fin in pend_fin:
            fin()
        pend_fin.clear()
        for fl in outproj_fillers(B - 1, N_QC - 1, tail=True):
            fl()
    drive()


def _rope_tables():
    """cos/sin tables exactly matching the reference's indexing quirk."""
    inv_freq = (1.0 / (ROPE_BASE ** (np.arange(0, HD, 2, dtype=np.float32) / HD)))
    t = np.arange(S, dtype=np.float32)
    freqs = np.outer(t, inv_freq)                       # [S, 64]
    emb = np.concatenate([freqs, freqs], axis=1)        # [S, 128]
    cos_part = np.cos(emb)[:, ::2]                      # [S, 64]
    sin_part = np.sin(emb)[:, 1::2]                     # [S, 64]
    # COS[d, s] = cos_part[s, d // 2]
    cos = cos_part.T[np.repeat(np.arange(HD // 2), 2)]  # [128, S]
    sin = sin_part.T[np.repeat(np.arange(HD // 2), 2)]
    return np.ascontiguousarray(cos), np.ascontiguousarray(sin)


def _pack_dtile_major(wt):
    """[D, E] (d, e) -> [128, N_DT * E]: row p holds [t, e] contiguously."""
    d, e = wt.shape
    return np.ascontiguousarray(
        wt.reshape(d // P, P, e).transpose(1, 0, 2).reshape(P, (d // P) * e))


def _host_prep(x, wq, wk, wv, wo):
    """Build the per-core input maps (SBUF-layout packed, bf16)."""
    bf = BF_NP
    xt = x.reshape(BS, D).T.astype(bf)                  # [D, BS]
    # pack to [N_SC, 128, N_DT*SC]: chunk sc, partition p -> (t, s) contiguous
    xt = np.ascontiguousarray(
        xt.reshape(N_DT, P, N_SC, SC).transpose(2, 1, 0, 3).reshape(
            N_SC, P, N_DT * SC))
    cos, sin = _rope_tables()
    cos = cos.astype(bf)
    sin = sin.astype(bf)
    rmat = np.zeros((P, P), dtype=np.float32)           # R^T for rot = R @ q
    idx = np.arange(0, P, 2)
    rmat[idx + 1, idx] = -1.0                           # R^T[2j+1, 2j] = -1
    rmat[idx, idx + 1] = 1.0                            # R^T[2j, 2j+1] = +1
    rmat = rmat.astype(bf)
    tri = np.triu(np.ones((P, P), dtype=np.float32)).astype(bf)

    in_maps = []
    for c in range(N_CORES):
        lo, hi = c * E, (c + 1) * E
        in_maps.append({
            "xt": xt,
            "wqt": _pack_dtile_major(wq[lo:hi].T.astype(bf)),
            "wkt": _pack_dtile_major(wk[lo:hi].T.astype(bf)),
            "wvt": _pack_dtile_major(wv[lo:hi].T.astype(bf)),
            "wot": _pack_dtile_major(wo[:, lo:hi].T.astype(bf)),
            "cos": cos,
            "sin": sin,
            "rmat": rmat,
            "tri": tri,
        })
    return in_maps


_CACHE = {}


def _get_program():
    if "nc" not in _CACHE:
        _CACHE["nc"] = _build_program()
    return _CACHE["nc"]


def _run(in_maps):
    from concourse.bass_utils import run_bass_kernel_spmd
    nc = _get_program()
    res = run_bass_kernel_spmd(nc, in_maps, core_ids=list(range(N_CORES)))
    return res


def kernel(x, wq, wk, wv, wo, attn_mask=None, **_):
    x = np.asarray(x, dtype=np.float32)
    in_maps = _host_prep(np.asarray(x, np.float32), np.asarray(wq, np.float32),
                         np.asarray(wk, np.float32), np.asarray(wv, np.float32),
                         np.asarray(wo, np.float32))
    res = _run(in_maps)
    out = np.zeros((BS, D), dtype=np.float32)
    for c in range(N_CORES):
        out += np.asarray(res.results[c]["out"], dtype=np.float32)
    return out.reshape(B, S, D)


if __name__ == "__main__":
    t0 = time.time()
    _get_program()
    print(f"program build: {time.time() - t0:.1f}s")

